# revision 1
# baseline (speedup 1.0000x reference)
"""Distributed 2-layer GAT on 8 Trainium2 NeuronCores (Bass/Tile).

Strategy (graph/data parallel, per sharding hint):
  - Nodes are sharded across 8 cores (6250 each, padded to 6272 = 49*128).
  - Within a core, nodes are greedily packed into 49 tiles of 128 so that
    per-tile in-edge counts are balanced (pad edges fill to K*128).
  - Layer tables ([h | al_src] per node) are computed locally per shard
    (x @ [W1 | W1@a_src | W1@a_dst] extended matmul) and AllGathered so
    every core holds the full node table in DRAM.
  - Edge pass per destination tile: indirect-DMA gather of source rows,
    attention weights ex = exp(leaky_relu(al_src[src]+al_dst[dst]))
    (unnormalized softmax - mathematically identical to the max-subtracted
    reference), weighted rows scatter-added into PSUM via a one-hot
    selection matmul; appended ex columns accumulate the softmax
    denominators in the same matmul. Per-node divide afterwards.
  - Dummy (pad) edges point their al_dst gather at a -1e9 row -> ex == 0.
"""

import heapq
import os
import sys
import types

import numpy as np

_BUILD_CACHE = {}


def _register_trace_hook():
    """Best-effort registration of the axon NTFF profiling hook."""
    try:
        if "antenv.axon_hooks" in sys.modules:
            return True
        from trn_agent_boot.trn_boot import _ntff_profile_via_ctypes

        hook = _ntff_profile_via_ctypes("/opt/axon/libaxon_pjrt.so")
        m = types.ModuleType("antenv.axon_hooks")
        m.get_axon_ntff_profile_hook = lambda: hook
        m.set_axon_ntff_profile_hook = lambda h: None
        sys.modules["antenv.axon_hooks"] = m
        return True
    except Exception:
        return False


def _host_prep(x, edge_index, W1, a_src1, a_dst1, b1, W2, a_src2, a_dst2, b2, C):
    x = np.asarray(x, np.float32)
    ei = np.asarray(edge_index)
    W1 = np.asarray(W1, np.float32)
    a_src1 = np.asarray(a_src1, np.float32)
    a_dst1 = np.asarray(a_dst1, np.float32)
    b1 = np.asarray(b1, np.float32)
    W2 = np.asarray(W2, np.float32)
    a_src2 = np.asarray(a_src2, np.float32)
    a_dst2 = np.asarray(a_dst2, np.float32)
    b2 = np.asarray(b2, np.float32)

    N, F = x.shape
    HEADS, HID = a_src1.shape
    D1 = HEADS * HID
    NCLS = W2.shape[1]
    assert N % C == 0
    NPC = N // C
    NT = -(-NPC // 128)
    PADN = NT * 128
    assert PADN > NPC, "need at least one pad slot per core for dummy rows"
    NPALL = C * PADN
    E = ei.shape[1]
    ET = E + N

    src = np.concatenate([ei[0], np.arange(N)]).astype(np.int64)
    dst = np.concatenate([ei[1], np.arange(N)]).astype(np.int64)

    # --- greedy degree-balanced node->tile assignment per core ---
    deg = np.bincount(dst, minlength=N).astype(np.int64)
    pos = np.empty(N, np.int64)
    for c in range(C):
        lo = c * NPC
        d = deg[lo:lo + NPC]
        order = np.argsort(-d, kind="stable")
        cnts = np.zeros(NT, np.int64)
        heap = [(0, t) for t in range(NT)]
        heapq.heapify(heap)
        ploc = np.empty(NPC, np.int64)
        for i in order:
            while True:
                load, t = heapq.heappop(heap)
                if cnts[t] < 128:
                    break
            ploc[i] = t * 128 + cnts[t]
            cnts[t] += 1
            if cnts[t] < 128:
                heapq.heappush(heap, (load + d[i], t))
        pos[lo:lo + NPC] = ploc

    ncidx = np.arange(N) // NPC
    node_at = np.full((C, PADN), -1, np.int64)
    node_at[ncidx, pos] = np.arange(N)
    grow = ncidx * PADN + pos  # global table row per node

    padrow = np.empty(C, np.int64)
    for c in range(C):
        w = np.where(node_at[c] < 0)[0]
        padrow[c] = c * PADN + w[0]

    # --- edge grouping by (dst core, dst tile) ---
    ec = dst // NPC
    et = pos[dst] // 128
    gkey = ec * NT + et
    # within each (core, tile) group, order edges by source table row so each
    # gather call's descriptors hit monotonically increasing DRAM addresses
    order_e = np.lexsort((pos[src] + (src // NPC) * PADN, gkey))
    ks = gkey[order_e]
    gstart = np.searchsorted(ks, np.arange(C * NT))
    gcnt = np.searchsorted(ks, np.arange(C * NT) + 1) - gstart
    K = int(-(-gcnt.max() // 128))
    jj = np.arange(ET) - gstart[ks]
    kk_e = jj // 128
    pp_e = jj % 128
    cc_e = ks // NT
    tt_e = ks % NT

    s_src = src[order_e]
    s_dst = dst[order_e]
    srcg = np.empty((C, NT, 128, K), np.int32)
    srcg[...] = padrow[:, None, None, None]
    dstl = np.full((C, NT, 128, K), 999.0, np.float32)
    srcg[cc_e, tt_e, pp_e, kk_e] = grow[s_src]
    dstl[cc_e, tt_e, pp_e, kk_e] = (pos[s_dst] % 128).astype(np.float32)

    # --- per-core transposed x shards (pad rows zero) ---
    xs = np.zeros((C, PADN, F), np.float32)
    xs[ncidx, pos] = x
    xsT = np.ascontiguousarray(xs.transpose(0, 2, 1))

    # --- extended weights ---
    Wa_s1 = np.einsum("fhc,hc->fh", W1.reshape(F, HEADS, HID), a_src1)
    Wa_d1 = np.einsum("fhc,hc->fh", W1.reshape(F, HEADS, HID), a_dst1)
    W1e = np.ascontiguousarray(
        np.concatenate([W1, Wa_s1, Wa_d1], axis=1), dtype=np.float32
    )
    Wa_s2 = W2 @ a_src2[0]
    Wa_d2 = W2 @ a_dst2[0]
    W2e = np.ascontiguousarray(
        np.concatenate([W2, Wa_s2[:, None], Wa_d2[:, None]], axis=1),
        dtype=np.float32,
    )

    # replicated-transposed dstl for the PE al_dst broadcast: [t, k, j, e] = dstl[t, e, k]
    dstlr = np.ascontiguousarray(
        np.broadcast_to(dstl.transpose(0, 1, 3, 2)[:, :, :, None, :],
                        (C, NT, K, 128, 128)), np.float32)
    iotac = np.arange(128, dtype=np.float32)[:, None].copy()
    b1r = np.ascontiguousarray(np.broadcast_to(b1[None, :], (128, D1)), np.float32)
    b2r = np.ascontiguousarray(np.broadcast_to(b2[None, :], (128, NCLS)), np.float32)
    iota = np.ascontiguousarray(
        np.broadcast_to(np.arange(128, dtype=np.float32)[None, :], (128, 128))
    )
    ident = np.eye(128, dtype=np.float32)

    cfg = dict(F=F, HEADS=HEADS, HID=HID, D1=D1, NCLS=NCLS, NT=NT, PADN=PADN,
               NPALL=NPALL, K=K, C=C)
    in_maps = []
    for c in range(C):
        in_maps.append({
            "xsT": xsT[c],
            "w1e": W1e,
            "w2e": W2e,
            "b1r": b1r,
            "b2r": b2r,
            "iota": iota,
            "ident": ident,
            "srcg": srcg[c],
            "dstl": dstl[c],
            "dstlr": dstlr[c],
            "iotac": iotac,
        })
    return cfg, in_maps, node_at, (N, NCLS)


def _build_program(F, HEADS, HID, D1, NCLS, NT, PADN, NPALL, K, C):
    import concourse.bacc as bacc
    import concourse.bass as bass
    import concourse.mybir as mybir
    import concourse.tile as tile

    f32 = mybir.dt.float32
    bf16 = mybir.dt.bfloat16
    i32 = mybir.dt.int32
    AF = mybir.ActivationFunctionType
    ALU = mybir.AluOpType
    AX = mybir.AxisListType

    TW1 = D1 + 2 * HEADS          # local layer-1 matmul width
    G1W = D1 + HEADS              # layer-1 gather row width [h | al_src]
    TW2 = NCLS + 2                # layer-2 table width [z2 | al_src2 | al_dst2]
    G2W = NCLS + 1                # layer-2 gather width [z2 | al_src2]
    FK = F // 128
    DK = D1 // 128

    nc = bacc.Bacc("TRN2", target_bir_lowering=False, debug=False, num_devices=C)

    xsT = nc.dram_tensor("xsT", [F, PADN], f32, kind="ExternalInput")
    w1e = nc.dram_tensor("w1e", [F, TW1], f32, kind="ExternalInput")
    w2e = nc.dram_tensor("w2e", [D1, TW2], f32, kind="ExternalInput")
    b1r = nc.dram_tensor("b1r", [128, D1], f32, kind="ExternalInput")
    b2r = nc.dram_tensor("b2r", [128, NCLS], f32, kind="ExternalInput")
    iot = nc.dram_tensor("iota", [128, 128], f32, kind="ExternalInput")
    idn = nc.dram_tensor("ident", [128, 128], f32, kind="ExternalInput")
    srcg = nc.dram_tensor("srcg", [NT, 128, K], i32, kind="ExternalInput")
    dstlr = nc.dram_tensor("dstlr", [NT, K, 128, 128], f32, kind="ExternalInput")
    iotac = nc.dram_tensor("iotac", [128, 1], f32, kind="ExternalInput")
    dstl = nc.dram_tensor("dstl", [NT, 128, K], f32, kind="ExternalInput")
    outp = nc.dram_tensor("outp", [PADN, NCLS], f32, kind="ExternalOutput")

    loc1 = nc.dram_tensor("loc1", [PADN, G1W], f32)
    tab1 = nc.dram_tensor("tab1", [NPALL, G1W], f32, addr_space="Shared")
    ald1 = nc.dram_tensor("ald1", [PADN + 1, HEADS], f32)
    loc2 = nc.dram_tensor("loc2", [PADN, TW2], f32)
    tab2 = nc.dram_tensor("tab2", [NPALL, TW2], f32, addr_space="Shared")
    ald2 = nc.dram_tensor("ald2", [PADN + 1, 1], f32)

    rg = [list(range(C))]

    with tile.TileContext(nc) as tc:
        with (
            tc.tile_pool(name="const", bufs=1) as const,
            tc.tile_pool(name="h2", bufs=1) as h2p,
            tc.tile_pool(name="shp", bufs=1) as shp,
            tc.tile_pool(name="wk", bufs=2) as wk,
            tc.tile_pool(name="idx", bufs=3) as idxp,
            tc.tile_pool(name="ps", bufs=2, space="PSUM") as psp,
        ):
            # ---- constants ----
            w1t = []
            for kk in range(FK):
                t_ = const.tile([128, TW1], f32, tag=f"w1_{kk}")
                nc.sync.dma_start(out=t_[:], in_=w1e[kk * 128:(kk + 1) * 128, :])
                w1t.append(t_)
            w2t = []
            for kk in range(DK):
                t_ = const.tile([128, TW2], f32, tag=f"w2_{kk}")
                nc.sync.dma_start(out=t_[:], in_=w2e[kk * 128:(kk + 1) * 128, :])
                w2t.append(t_)
            b1s = const.tile([128, D1], f32, tag="b1")
            nc.sync.dma_start(out=b1s[:], in_=b1r[:, :])
            b2s = const.tile([128, NCLS], f32, tag="b2")
            nc.sync.dma_start(out=b2s[:], in_=b2r[:, :])
            ios = const.tile([128, 128], f32, tag="iota")
            nc.sync.dma_start(out=ios[:], in_=iot[:, :])
            ids = const.tile([128, 128], f32, tag="ident")
            nc.sync.dma_start(out=ids[:], in_=idn[:, :])
            neg1 = const.tile([1, HEADS], f32, tag="neg1")
            nc.vector.memset(neg1[:], -1e9)
            nc.sync.dma_start(out=ald1[PADN:PADN + 1, :], in_=neg1[:])
            neg2 = const.tile([1, 1], f32, tag="neg2")
            nc.vector.memset(neg2[:], -1e9)
            nc.sync.dma_start(out=ald2[PADN:PADN + 1, :], in_=neg2[:])
            ioc = const.tile([128, 1], f32, tag="ioc")
            nc.sync.dma_start(out=ioc[:], in_=iotac[:, :])
            ssum = const.tile([128, NT], f32, tag="ssum")
            lgs = const.tile([128, NT], f32, tag="lgs")

            # ---- phase A: local h = x @ [W1 | Wa_src | Wa_dst] ----
            with nc.named_scope("l1_local_mm"):
                for t in range(NT):
                    ps_a = psp.tile([128, TW1], f32, tag="mm")
                    for kk in range(FK):
                        xt = wk.tile([128, 128], f32, tag=f"xt{kk}")
                        nc.sync.dma_start(
                            out=xt[:],
                            in_=xsT[kk * 128:(kk + 1) * 128, t * 128:(t + 1) * 128],
                        )
                        nc.tensor.matmul(ps_a[:], lhsT=xt[:], rhs=w1t[kk][:],
                                         start=(kk == 0), stop=(kk == FK - 1))
                    ha = wk.tile([128, TW1], f32, tag="ha")
                    nc.scalar.copy(ha[:], ps_a[:])
                    nc.sync.dma_start(out=loc1[t * 128:(t + 1) * 128, :],
                                      in_=ha[:, 0:G1W])
                    nc.sync.dma_start(out=ald1[t * 128:(t + 1) * 128, :],
                                      in_=ha[:, D1 + HEADS:D1 + 2 * HEADS])

            # ---- phase B: allgather layer-1 table ----
            with nc.named_scope("l1_allgather"):
                nc.gpsimd.collective_compute(
                    "AllGather", mybir.AluOpType.bypass, replica_groups=rg,
                    ins=[loc1[:]], outs=[tab1[:]],
                )
            tc.strict_bb_all_engine_barrier()

            # ---- phase C: layer-1 edge pass ----
            o1_tiles = []
            with nc.named_scope("l1_edges"):
                for t in range(NT):
                    sg = idxp.tile([128, K], i32, tag="sg")
                    nc.sync.dma_start(out=sg[:], in_=srcg[t])
                    dl = idxp.tile([128, K], f32, tag="dl")
                    nc.sync.dma_start(out=dl[:], in_=dstl[t])
                    alt = idxp.tile([128, HEADS], f32, tag="alt")
                    nc.sync.dma_start(out=alt[:], in_=ald1[t * 128:(t + 1) * 128, :])

                    g1 = wk.tile([128, K * G1W], f32, tag="g1")
                    g1v = g1[:].rearrange("p (k c) -> p k c", c=G1W)
                    for k in range(K):
                        nc.gpsimd.indirect_dma_start(
                            out=g1v[:, k, :], out_offset=None, in_=tab1[:, :],
                            in_offset=bass.IndirectOffsetOnAxis(
                                ap=sg[:, k:k + 1], axis=0),
                        )
                    # al_dst per edge via transposed-onehot matmul
                    ps_e = psp.tile([128, K * HEADS], f32, tag="ed")
                    for k in range(K):
                        dr = wk.tile([128, 128], f32, tag="dr")
                        nc.sync.dma_start(out=dr[:], in_=dstlr[t, k])
                        ohT = wk.tile([128, 128], f32, tag="ohT")
                        nc.vector.tensor_scalar(ohT[:], dr[:], ioc[:, 0:1], None,
                                                op0=ALU.is_equal)
                        nc.tensor.matmul(ps_e[:, k * HEADS:(k + 1) * HEADS],
                                         lhsT=ohT[:], rhs=alt[:],
                                         start=(k == 0), stop=(k == K - 1))

                    e1 = idxp.tile([128, K * HEADS], f32, tag="e1")
                    e1v = e1[:].rearrange("p (k h) -> p k h", h=HEADS)
                    nc.vector.tensor_add(
                        e1v, g1v[:, :, D1:D1 + HEADS],
                        ps_e[:].rearrange("p (k h) -> p k h", h=HEADS))
                    lr = idxp.tile([128, K * HEADS], f32, tag="lr")
                    nc.vector.tensor_scalar_mul(lr[:], e1[:], 0.2)
                    nc.vector.tensor_max(lr[:], lr[:], e1[:])
                    exw = idxp.tile([128, K * HEADS], f32, tag="exw")
                    nc.scalar.activation(exw[:], lr[:], AF.Exp)
                    exv = exw[:].rearrange("p (k h) -> p k h", h=HEADS)

                    # weight gathered rows in place; ex into the al_src cols
                    g1f = g1v[:, :, 0:D1].rearrange("p k (h c) -> p k h c", c=HID)
                    exb = exv.unsqueeze(3).to_broadcast([128, K, HEADS, HID])
                    nc.vector.tensor_mul(g1f, g1f, exb)
                    nc.vector.tensor_copy(g1v[:, :, D1:D1 + HEADS], exv)

                    oh = wk.tile([128, K * 128], f32, tag="oh")
                    ohv = oh[:].rearrange("p (k j) -> p k j", j=128)
                    dlb = dl[:].unsqueeze(2).to_broadcast([128, K, 128])
                    iob = ios[:].unsqueeze(1).to_broadcast([128, K, 128])
                    nc.vector.tensor_tensor(ohv, dlb, iob, op=ALU.is_equal)

                    ps_c = psp.tile([128, TW1], f32, tag="mm")
                    for k in range(K):
                        nc.tensor.matmul(
                            ps_c[:, 0:G1W],
                            lhsT=oh[:, k * 128:(k + 1) * 128],
                            rhs=g1[:, k * G1W:(k + 1) * G1W],
                            start=(k == 0), stop=(k == K - 1),
                        )

                    den = idxp.tile([128, HEADS], f32, tag="den")
                    nc.vector.tensor_scalar_add(den[:], ps_c[:, D1:D1 + HEADS], 1e-16)
                    rec = idxp.tile([128, HEADS], f32, tag="rec")
                    nc.vector.reciprocal(rec[:], den[:])

                    o1 = h2p.tile([128, D1], f32, tag=f"h2_{t}")
                    o1v = o1[:].rearrange("p (h c) -> p h c", c=HID)
                    recb = rec[:].unsqueeze(2).to_broadcast([128, HEADS, HID])
                    psf = ps_c[:, 0:D1].rearrange("p (h c) -> p h c", c=HID)
                    nc.vector.tensor_mul(o1v, psf, recb)
                    nc.vector.tensor_add(o1[:], o1[:], b1s[:])
                    # elu(x) = max(x,0) + exp(min(x,0)) - 1
                    tn = wk.tile([128, D1], f32, tag="tn")
                    nc.vector.tensor_scalar_min(tn[:], o1[:], 0.0)
                    nc.scalar.activation(tn[:], tn[:], AF.Exp)
                    nc.vector.tensor_scalar_max(o1[:], o1[:], 0.0)
                    nc.vector.tensor_add(o1[:], o1[:], tn[:])
                    nc.vector.tensor_scalar_add(o1[:], o1[:], -1.0)
                    o1_tiles.append(o1)

            # ---- phase D: layer-2 local z2 = h2 @ [W2 | Wa2_src | Wa2_dst] ----
            with nc.named_scope("l2_local_mm"):
                for t in range(NT):
                    tts = []
                    for kk in range(DK):
                        ps_t = psp.tile([128, 128], f32, tag="tr")
                        nc.tensor.transpose(
                            ps_t[:], o1_tiles[t][:, kk * 128:(kk + 1) * 128], ids[:]
                        )
                        tt = wk.tile([128, 128], f32, tag=f"tt{kk}")
                        nc.scalar.copy(tt[:], ps_t[:])
                        tts.append(tt)
                    ps_d = psp.tile([128, TW2], f32, tag="mm")
                    for kk in range(DK):
                        nc.tensor.matmul(ps_d[:], lhsT=tts[kk][:], rhs=w2t[kk][:],
                                         start=(kk == 0), stop=(kk == DK - 1))
                    hd = wk.tile([128, TW2], f32, tag="hd")
                    nc.scalar.copy(hd[:], ps_d[:])
                    nc.sync.dma_start(out=loc2[t * 128:(t + 1) * 128, :],
                                      in_=hd[:, 0:TW2])
                    nc.sync.dma_start(out=ald2[t * 128:(t + 1) * 128, :],
                                      in_=hd[:, TW2 - 1:TW2])

            # ---- phase E: allgather layer-2 table ----
            with nc.named_scope("l2_allgather"):
                nc.gpsimd.collective_compute(
                    "AllGather", mybir.AluOpType.bypass, replica_groups=rg,
                    ins=[loc2[:]], outs=[tab2[:]],
                )
            tc.strict_bb_all_engine_barrier()

            # ---- phase F: layer-2 edge pass ----
            sh_tiles = []
            with nc.named_scope("l2_edges"):
                for t in range(NT):
                    sg = idxp.tile([128, K], i32, tag="sg")
                    nc.sync.dma_start(out=sg[:], in_=srcg[t])
                    dl = idxp.tile([128, K], f32, tag="dl")
                    nc.sync.dma_start(out=dl[:], in_=dstl[t])
                    alt2 = idxp.tile([128, 1], f32, tag="alt2")
                    nc.sync.dma_start(out=alt2[:], in_=ald2[t * 128:(t + 1) * 128, :])

                    g2 = wk.tile([128, K * G2W], f32, tag="g2")
                    g2v = g2[:].rearrange("p (k c) -> p k c", c=G2W)
                    for k in range(K):
                        nc.gpsimd.indirect_dma_start(
                            out=g2v[:, k, :], out_offset=None, in_=tab2[:, :],
                            in_offset=bass.IndirectOffsetOnAxis(
                                ap=sg[:, k:k + 1], axis=0),
                        )
                    ps_e2 = psp.tile([128, K * HEADS], f32, tag="ed")
                    for k in range(K):
                        dr = wk.tile([128, 128], f32, tag="dr")
                        nc.sync.dma_start(out=dr[:], in_=dstlr[t, k])
                        ohT = wk.tile([128, 128], f32, tag="ohT")
                        nc.vector.tensor_scalar(ohT[:], dr[:], ioc[:, 0:1], None,
                                                op0=ALU.is_equal)
                        nc.tensor.matmul(ps_e2[:, k:k + 1],
                                         lhsT=ohT[:], rhs=alt2[:],
                                         start=(k == 0), stop=(k == K - 1))

                    e2 = idxp.tile([128, K], f32, tag="e2")
                    nc.vector.tensor_add(e2[:], g2v[:, :, NCLS], ps_e2[:, 0:K])
                    lr2 = idxp.tile([128, K], f32, tag="lr2")
                    nc.vector.tensor_scalar_mul(lr2[:], e2[:], 0.2)
                    nc.vector.tensor_max(lr2[:], lr2[:], e2[:])
                    ex2 = idxp.tile([128, K], f32, tag="ex2")
                    nc.scalar.activation(ex2[:], lr2[:], AF.Exp)

                    g2f = g2v[:, :, 0:NCLS]
                    ex2b = ex2[:].unsqueeze(2).to_broadcast([128, K, NCLS])
                    nc.vector.tensor_mul(g2f, g2f, ex2b)
                    nc.vector.tensor_copy(g2v[:, :, NCLS], ex2[:])

                    oh = wk.tile([128, K * 128], f32, tag="oh")
                    ohv = oh[:].rearrange("p (k j) -> p k j", j=128)
                    dlb = dl[:].unsqueeze(2).to_broadcast([128, K, 128])
                    iob = ios[:].unsqueeze(1).to_broadcast([128, K, 128])
                    nc.vector.tensor_tensor(ohv, dlb, iob, op=ALU.is_equal)

                    ps_f = psp.tile([128, TW1], f32, tag="mm")
                    for k in range(K):
                        nc.tensor.matmul(
                            ps_f[:, 0:G2W],
                            lhsT=oh[:, k * 128:(k + 1) * 128],
                            rhs=g2[:, k * G2W:(k + 1) * G2W],
                            start=(k == 0), stop=(k == K - 1),
                        )

                    den2 = idxp.tile([128, 1], f32, tag="den2")
                    nc.vector.tensor_scalar_add(den2[:], ps_f[:, NCLS:NCLS + 1], 1e-16)
                    rec2 = idxp.tile([128, 1], f32, tag="rec2")
                    nc.vector.reciprocal(rec2[:], den2[:])

                    o2 = wk.tile([128, NCLS], f32, tag="o2")
                    nc.vector.tensor_scalar(o2[:], ps_f[:, 0:NCLS], rec2[:], None,
                                            op0=ALU.mult)
                    nc.vector.tensor_add(o2[:], o2[:], b2s[:])

                    rmax = idxp.tile([128, 1], f32, tag="rmax")
                    nc.vector.reduce_max(rmax[:], o2[:], axis=AX.X)
                    sh = shp.tile([128, NCLS], f32, tag=f"sh_{t}")
                    nc.vector.tensor_scalar(sh[:], o2[:], rmax[:], None,
                                            op0=ALU.subtract)
                    exs = wk.tile([128, NCLS], f32, tag="exs")
                    nc.scalar.activation(exs[:], sh[:], AF.Exp)
                    nc.vector.reduce_sum(ssum[:, t:t + 1], exs[:], axis=AX.X)
                    sh_tiles.append(sh)

            # ---- phase G: log-softmax finalize ----
            with nc.named_scope("logsoftmax"):
                nc.scalar.activation(lgs[:], ssum[:], AF.Ln)
                for t in range(NT):
                    outf = wk.tile([128, NCLS], f32, tag="outf")
                    nc.vector.tensor_scalar(outf[:], sh_tiles[t][:], lgs[:, t:t + 1],
                                            None, op0=ALU.subtract)
                    nc.sync.dma_start(out=outp[t * 128:(t + 1) * 128, :], in_=outf[:])

    nc.compile()
    return nc


def _get_program(cfg):
    key = tuple(sorted(cfg.items()))
    if key not in _BUILD_CACHE:
        _BUILD_CACHE[key] = _build_program(**cfg)
    return _BUILD_CACHE[key]


def kernel(**inputs):
    C = 8
    cfg, in_maps, node_at, (N, NCLS) = _host_prep(
        inputs["x"], inputs["edge_index"], inputs["W1"], inputs["a_src1"],
        inputs["a_dst1"], inputs["b1"], inputs["W2"], inputs["a_src2"],
        inputs["a_dst2"], inputs["b2"], C,
    )
    nc = _get_program(cfg)

    from concourse.bass_utils import run_bass_kernel_spmd

    trace = bool(int(os.environ.get("GAT_PROFILE", "0")))
    if trace:
        trace = _register_trace_hook()
    res = run_bass_kernel_spmd(nc, in_maps, list(range(C)), trace=trace)
    if trace and res.exec_time_ns is not None:
        print(f"HW exec time: {res.exec_time_ns} ns", flush=True)

    out = np.empty((N, NCLS), np.float32)
    for c in range(C):
        r = res.results[c]["outp"]
        m = node_at[c] >= 0
        out[node_at[c][m]] = r[m]
    return out



# revision 19
# speedup vs baseline: 2.9148x; 2.9148x over previous
"""Distributed 2-layer GAT on 8 Trainium2 NeuronCores (Bass/Tile), v7.

Strategy (graph/data parallel, dst-aligned edge grids):
  - Nodes sharded across 8 cores (6250 each, padded to 6272 = 49*128).
  - Per core, nodes are packed into 49 tiles of 128 by iterated (lo,hi)
    gather-load sorting, so each tile's max per-window in-degree (the
    edge-grid width K_t) stays tight; K unified across cores (SPMD).
  - Edge grids are DST-ALIGNED: partition i holds only edges whose
    destination is the tile's node at row i.  al_dst is a per-partition
    broadcast add, the softmax denominator a row reduce, the weighted
    scatter-sum K accumulating PE matmuls with a constant bf16 identity.
  - Node tables ([h | a_src] rows, bf16) are AllGathered in 7 chunks that
    pipeline with the producing compute, restrided into contiguous
    ExternalOutput buffers (the dma_gather ucode needs jax-allocated
    contiguous tables), then fetched per edge with one dma_gather per
    (tile, index-window).  int16 gather indices -> two overlapping 32768-
    row windows; edges from the 15360-row overlap balance the two calls.
  - Pad slots point at rows whose a_src columns hold -1e9 (exact 0 weight).
"""

import os
import sys
import types

import numpy as np

_BUILD_CACHE = {}

C = 8
N, F, HEADS, HID, NCLS = 50000, 512, 4, 64, 64
D1 = HEADS * HID        # 256
NPC = N // C            # 6250
NT = 49
PADN = NT * 128         # 6272
NPALL = C * PADN        # 50176
NCH = 7                 # allgather chunks (7 tiles each)
TPC = NT // NCH         # tiles per chunk
CROWS = TPC * 128       # 896 rows per (core, chunk)
CHB = C * CROWS         # 7168 rows per chunk block in the global table
HALF = 32768            # int16 index limit for dma_gather
HIBASE = NPALL - HALF   # hi-window base (17408); windows overlap 15360 rows
S1 = 384                # layer-1 gather-table row stride (bf16 cols; 768B)
G1 = D1 + HEADS         # gathered layer-1 row width: 260
S2 = 128                # layer-2 gather-table row stride (bf16 cols; 256B)
G2 = NCLS + 1           # gathered layer-2 row width: 65
TW1 = D1 + 2 * HEADS    # phase-A matmul width 264
TW2 = NCLS + 2          # phase-D matmul width 66
FK = F // 128           # 4
DK = D1 // 128          # 2
NEG = -1e9
PADROW0 = 127           # tile-0 pad row (every core; rank gap at 127)
PADROWL = (NPC + 1) % 128  # 107: first pad row within the last tile


def _grow(core, posn):
    """Global gather-table row for (core, in-core position): chunk-major."""
    t = posn // 128
    return (t // TPC) * CHB + core * CROWS + (t % TPC) * 128 + posn % 128


def _register_trace_hook():
    try:
        if "antenv.axon_hooks" in sys.modules:
            return True
        from trn_agent_boot.trn_boot import _ntff_profile_via_ctypes

        hook = _ntff_profile_via_ctypes("/opt/axon/libaxon_pjrt.so")
        m = types.ModuleType("antenv.axon_hooks")
        m.get_axon_ntff_profile_hook = lambda: hook
        m.set_axon_ntff_profile_hook = lambda h: None
        sys.modules["antenv.axon_hooks"] = m
        return True
    except Exception:
        return False


def _wrap16(flat):
    """dma_gather index layout: wrapped in 16 partitions, replicated x8."""
    n = len(flat)
    assert n % 16 == 0
    w = np.asarray(flat, np.int64).reshape(n // 16, 16).T.astype(np.int32)
    w = np.tile(w, (8, 1))
    return w.astype(np.uint16).view(np.int16)


def _host_prep(x, edge_index, W1, a_src1, a_dst1, b1, W2, a_src2, a_dst2, b2):
    import ml_dtypes
    bf = ml_dtypes.bfloat16

    x = np.asarray(x, np.float32)
    ei = np.asarray(edge_index)
    W1 = np.asarray(W1, np.float32)
    a_src1 = np.asarray(a_src1, np.float32)
    a_dst1 = np.asarray(a_dst1, np.float32)
    b1 = np.asarray(b1, np.float32)
    W2 = np.asarray(W2, np.float32)
    a_src2 = np.asarray(a_src2, np.float32)
    a_dst2 = np.asarray(a_dst2, np.float32)
    b2 = np.asarray(b2, np.float32)

    src = np.concatenate([ei[0], np.arange(N)]).astype(np.int64)
    dst = np.concatenate([ei[1], np.arange(N)]).astype(np.int64)
    deg = np.bincount(dst, minlength=N).astype(np.int64)
    ncidx = np.arange(N) // NPC

    def assign_rows(keys):
        """rank -> position, skipping position 127 (the lo-window pad row)."""
        p = np.empty(N, np.int64)
        for c in range(C):
            s = slice(c * NPC, (c + 1) * NPC)
            order = np.lexsort(tuple(k[s] for k in keys))
            q = np.empty(NPC, np.int64)
            r = np.arange(NPC)
            q[order] = r + (r >= PADROW0)
            p[s] = q
        return p

    def window_loads(pos_):
        grow_ = _grow(ncidx, pos_)
        gs_ = grow_[src]
        gd_ = grow_[dst]
        zone_ = (gs_ >= HIBASE).astype(np.int64) + (gs_ >= HALF).astype(np.int64)
        cnt = np.bincount(gd_ * 3 + zone_, minlength=NPALL * 3).reshape(NPALL, 3)
        n1, nf, n2 = cnt[:, 0], cnt[:, 1], cnt[:, 2]
        xf = np.clip((n2 + nf - n1 + 1) // 2, 0, nf)
        return n1 + xf, n2 + nf - xf

    pos = assign_rows((-deg,))
    for _ in range(4):
        wlo_t, whi_t = window_loads(pos)
        g_ = _grow(ncidx, pos)
        pos = assign_rows((-whi_t[g_], -wlo_t[g_]))

    grow = _grow(ncidx, pos)
    node_at = np.full((C, PADN), -1, np.int64)
    node_at[ncidx, pos] = np.arange(N)

    # --- per-(core,tile,row) edge grouping into overlapping lo/hi windows ---
    gs = grow[src]
    lrow = ncidx[dst] * PADN + pos[dst]              # (core, local row)
    zone = (gs >= HIBASE).astype(np.int64) + (gs >= HALF).astype(np.int64)
    okey = np.lexsort((gs, zone, lrow))
    gs_s = gs[okey]
    key_s = (lrow * 3 + zone)[okey]
    bounds = np.searchsorted(key_s, np.arange(NPALL * 3 + 1))
    d3 = (bounds[1:] - bounds[:-1]).reshape(NPALL, 3)
    n1, nf, n2 = d3[:, 0], d3[:, 1], d3[:, 2]
    xfl = np.clip((n2 + nf - n1 + 1) // 2, 0, nf)
    wlo = n1 + xfl
    whi = n2 + nf - xfl
    KLO = np.maximum(wlo.reshape(C, NT, 128).max(axis=2).max(axis=0), 1)
    KHI = np.maximum(whi.reshape(C, NT, 128).max(axis=2).max(axis=0), 1)

    PADLO = PADROW0                          # core-0 tile-0 row-127 pad
    PADHI = _grow(7, NPC + 1) - HIBASE       # core-7 last-tile pad, hi-local
    assert 0 <= PADHI < HALF, PADHI

    idx_lo = np.full((C, NT), None, dtype=object)
    idx_hi = np.full((C, NT), None, dtype=object)
    for c in range(C):
        for t in range(NT):
            klo, khi = KLO[t], KHI[t]
            glo = np.full((128, klo), PADLO, np.int64)
            ghi = np.full((128, khi), PADHI, np.int64)
            for i in range(128):
                r = (c * PADN + t * 128) + i
                b0, b1_, b2_, b3_ = bounds[3 * r:3 * r + 4]
                xl = xfl[r]
                lo_rows = np.concatenate([gs_s[b0:b1_], gs_s[b1_:b1_ + xl]])
                hi_rows = gs_s[b1_ + xl:b3_]
                glo[i, :len(lo_rows)] = lo_rows
                ghi[i, :len(hi_rows)] = hi_rows - HIBASE
                if b3_ == b0:
                    # padded (node-less) row: one REAL gather keeps its
                    # softmax denominator nonzero (no eps on device)
                    glo[i, 0] = 0
            idx_lo[c, t] = glo.T.ravel()   # column-major = placement order
            idx_hi[c, t] = ghi.T.ravel()

    # --- per-core transposed x shards, bf16, per-tile-contiguous blocks ---
    xs = np.zeros((C, PADN, F), np.float32)
    xs[ncidx, pos] = x
    xsTt = np.ascontiguousarray(
        xs.reshape(C, NT, 128, F).transpose(0, 1, 3, 2)).astype(bf)  # [C,NT,F,128]

    # --- extended weights (bf16) ---
    Wa_s1 = np.einsum("fhc,hc->fh", W1.reshape(F, HEADS, HID), a_src1)
    Wa_d1 = np.einsum("fhc,hc->fh", W1.reshape(F, HEADS, HID), a_dst1)
    W1e = np.ascontiguousarray(
        np.concatenate([W1, Wa_s1, Wa_d1], axis=1)).astype(bf)       # [512,264]
    Wa_s2 = W2 @ a_src2[0]
    Wa_d2 = W2 @ a_dst2[0]
    W2e = np.ascontiguousarray(np.concatenate(
        [W2, Wa_s2[:, None], Wa_d2[:, None]], axis=1)).astype(bf)    # [256,66]
    b1r = np.broadcast_to(b1[None, :], (128, D1)).astype(np.float32).copy()
    b2r = np.broadcast_to(b2[None, :], (128, NCLS)).astype(np.float32).copy()
    ident = np.eye(128, dtype=np.float32).astype(bf)
    padm = np.zeros((128, 2 * HEADS), np.float32)
    padm[PADROW0, 0:HEADS] = NEG           # tile-0 pad row mask
    padm[PADROWL:, HEADS:2 * HEADS] = NEG  # last-tile pad rows mask

    in_maps = []
    for c in range(C):
        in_maps.append({
            "xsTt": xsTt[c],
            "w1e": W1e,
            "w2e": W2e,
            "b1r": b1r,
            "b2r": b2r,
            "ident": np.ascontiguousarray(ident),
            "ilo": np.hstack([_wrap16(idx_lo[c, t]) for t in range(NT)]),
            "ihi": np.hstack([_wrap16(idx_hi[c, t]) for t in range(NT)]),
            "padm": padm,
        })
    cfg = dict(KLO=tuple(int(v) for v in KLO), KHI=tuple(int(v) for v in KHI))
    return cfg, in_maps, node_at


def _dma_gather_raw(nc, out_ap, in_ap, idxs_ap, num_idxs, elem_size, elem_step,
                    queue_num=0):
    """nc.gpsimd.dma_gather minus the elem_size%256 over-assert (the ISA only
    requires the ROW STRIDE to be a 256B multiple; verified on hardware)."""
    import concourse.mybir as mybir
    from concourse.bass import exact_div

    g = nc.gpsimd
    stride_bytes = elem_step * mybir.dt.size(in_ap.dtype)
    stride_bytes_256 = exact_div(stride_bytes, 256)
    _in_ap = g.lower_ap_dma(in_ap, for_custom_bir_dma=True)
    _idxs_ap = g.lower_ap(idxs_ap)
    _out_ap = g.lower_ap(out_ap)
    return g.add_instruction(
        mybir.InstDMAGatherAnt(
            name=g.bass.get_next_instruction_name(),
            ins=[*_in_ap, _idxs_ap, g.lower_val_access(g.to_reg(num_idxs))],
            outs=[_out_ap],
            transpose=False,
            num_idxs=num_idxs,
            elem_size=elem_size,
            stride_bytes_256=stride_bytes_256,
            gen_mode=0,
            single_packet=False,
            queue_num=queue_num,
            sbuf_tokens_per_rank=0,
            sbuf_free_dim_per_rank=0,
            sbuf_free_dim_pad_per_rank=0,
            sbuf_byte_offset=0,
        )
    )


def _build_program(KLO, KHI):
    import concourse.bacc as bacc
    import concourse.bass as bass
    import concourse.mybir as mybir
    import concourse.tile as tile

    f32 = mybir.dt.float32
    bf16 = mybir.dt.bfloat16
    i16 = mybir.dt.int16
    AF = mybir.ActivationFunctionType
    ALU = mybir.AluOpType
    AX = mybir.AxisListType

    KLO = list(KLO)
    KHI = list(KHI)
    K = [a + b for a, b in zip(KLO, KHI)]
    LOFF = np.concatenate([[0], np.cumsum(KLO)]).tolist()
    HOFF = np.concatenate([[0], np.cumsum(KHI)]).tolist()

    NSWQ = int(os.environ.get("GAT_NSWQ", "4"))
    nc = bacc.Bacc("TRN2", target_bir_lowering=False, debug=False,
                   num_devices=C, num_swdge_queues=NSWQ)

    xsTt = nc.dram_tensor("xsTt", [NT, F, 128], bf16, kind="ExternalInput")
    w1e = nc.dram_tensor("w1e", [F, TW1], bf16, kind="ExternalInput")
    w2e = nc.dram_tensor("w2e", [D1, TW2], bf16, kind="ExternalInput")
    b1r = nc.dram_tensor("b1r", [128, D1], f32, kind="ExternalInput")
    b2r = nc.dram_tensor("b2r", [128, NCLS], f32, kind="ExternalInput")
    idn = nc.dram_tensor("ident", [128, 128], bf16, kind="ExternalInput")
    padm = nc.dram_tensor("padm", [128, 2 * HEADS], f32, kind="ExternalInput")
    ilo = nc.dram_tensor("ilo", [128, LOFF[NT] * 8], i16, kind="ExternalInput")
    ihi = nc.dram_tensor("ihi", [128, HOFF[NT] * 8], i16, kind="ExternalInput")
    outp = nc.dram_tensor("outp", [PADN, NCLS], f32, kind="ExternalOutput")

    loc1 = [nc.dram_tensor(f"loc1_{j}", [CROWS, G1], bf16) for j in range(NCH)]
    t1i = [nc.dram_tensor(f"tab1i_{j}", [CHB, G1], bf16, addr_space="Shared")
           for j in range(NCH)]
    tab1 = nc.dram_tensor("tabg1", [NPALL, S1], bf16, kind="ExternalOutput")
    loc2 = [nc.dram_tensor(f"loc2_{j}", [CROWS, S2], bf16) for j in range(NCH)]
    t2i = [nc.dram_tensor(f"tab2i_{j}", [CHB, S2], bf16, addr_space="Shared")
           for j in range(NCH)]
    tab2 = nc.dram_tensor("tabg2", [NPALL, S2], bf16, kind="ExternalOutput")

    rg = [list(range(C))]

    with tile.TileContext(nc) as tc:
        with (
            tc.tile_pool(name="const", bufs=1) as const,
            tc.tile_pool(name="wk", bufs=4) as wk,
            tc.tile_pool(name="gv", bufs=4) as gvp,
            tc.tile_pool(name="ps", bufs=2, space="PSUM") as psp,
            tc.tile_pool(name="pst", bufs=2, space="PSUM") as pstp,
        ):
            # ---- constants ----
            w1s = const.tile([128, FK * TW1], bf16, tag="w1")
            nc.sync.dma_start(
                out=w1s[:].rearrange("p (k c) -> p k c", c=TW1),
                in_=w1e[:, :].rearrange("(k p) c -> p k c", p=128),
            )
            w2s = const.tile([128, DK * TW2], bf16, tag="w2")
            nc.sync.dma_start(
                out=w2s[:].rearrange("p (k c) -> p k c", c=TW2),
                in_=w2e[:, :].rearrange("(k p) c -> p k c", p=128),
            )
            b1s = const.tile([128, D1], f32, tag="b1")
            nc.sync.dma_start(out=b1s[:], in_=b1r[:, :])
            b2s = const.tile([128, NCLS], f32, tag="b2")
            nc.sync.dma_start(out=b2s[:], in_=b2r[:, :])
            ids = const.tile([128, 128], bf16, tag="ident")
            nc.sync.dma_start(out=ids[:], in_=idn[:, :])
            padms = const.tile([128, 2 * HEADS], f32, tag="padm")
            nc.sync.dma_start(out=padms[:], in_=padm[:, :])
            ald1s = const.tile([128, NT * HEADS], f32, tag="ald1")
            ald2s = const.tile([128, NT], f32, tag="ald2")
            zc = const.tile([128, 1], f32, tag="zc")
            nc.vector.memset(zc[:], 0.0)
            ilos = const.tile([128, LOFF[NT] * 8], i16, tag="ilos")
            nc.sync.dma_start(out=ilos[:], in_=ilo[:, :])
            ihis = const.tile([128, HOFF[NT] * 8], i16, tag="ihis")
            nc.sync.dma_start(out=ihis[:], in_=ihi[:, :])
            shs = const.tile([128, NT * NCLS], f32, tag="shs")
            ssums = const.tile([128, NT], f32, tag="ssums")
            lgss = const.tile([128, NT], f32, tag="lgss")

            # ---- phase A: local table 1, chunk-pipelined with AG1+copy ----
            with nc.named_scope("l1_local_mm"):
                for t in range(NT):
                    j, tj = t // TPC, t % TPC
                    xa = wk.tile([128, F], bf16, tag="xa")
                    nc.sync.dma_start(
                        out=xa[:].rearrange("p (k n) -> p k n", n=128),
                        in_=xsTt[t].rearrange("(k p) n -> p k n", p=128),
                    )
                    ps_a = psp.tile([128, TW1], f32, tag="mm")
                    for kk in range(FK):
                        nc.tensor.matmul(
                            ps_a[:],
                            lhsT=xa[:, kk * 128:(kk + 1) * 128],
                            rhs=w1s[:, kk * TW1:(kk + 1) * TW1],
                            start=(kk == 0), stop=(kk == FK - 1),
                        )
                    ha = wk.tile([128, G1], bf16, tag="ha")
                    nc.vector.tensor_copy(ha[:], ps_a[:, 0:G1])
                    if t == 0:
                        nc.vector.tensor_add(ha[:, D1:G1], ha[:, D1:G1],
                                             padms[:, 0:HEADS])
                    if t == NT - 1:
                        nc.vector.tensor_add(ha[:, D1:G1], ha[:, D1:G1],
                                             padms[:, HEADS:2 * HEADS])
                    nc.sync.dma_start(
                        out=loc1[j][tj * 128:(tj + 1) * 128, :], in_=ha[:])
                    nc.vector.tensor_copy(
                        ald1s[:, t * HEADS:(t + 1) * HEADS],
                        ps_a[:, G1:G1 + HEADS])
                    if tj == TPC - 1:
                        with nc.named_scope("l1_ag"):
                            nc.gpsimd.collective_compute(
                                "AllGather", mybir.AluOpType.bypass,
                                replica_groups=rg,
                                ins=[loc1[j][:]], outs=[t1i[j][:]],
                            )
                        with nc.named_scope("l1_restride"):
                            nc.sync.dma_start(
                                out=tab1[j * CHB:(j + 1) * CHB, 0:G1],
                                in_=t1i[j][:])

            # ---- phase C: layer-1 edge pass (+ fused layer-2 local mm) ----
            with nc.named_scope("l1_edges"):
                for t in range(NT):
                    j, tj = t // TPC, t % TPC
                    klo, khi, kt = KLO[t], KHI[t], K[t]
                    g = gvp.tile([128, kt * G1], bf16, tag="g1")
                    gv = g[:].rearrange("p (k c) -> p k c", c=G1)
                    _dma_gather_raw(nc, gv[:, 0:klo, :], tab1[0:HALF, 0:G1],
                                    ilos[:, LOFF[t] * 8:LOFF[t + 1] * 8],
                                    klo * 128, G1, S1,
                                    queue_num=(2 * t) % NSWQ)
                    _dma_gather_raw(nc, gv[:, klo:kt, :],
                                    tab1[HIBASE:NPALL, 0:G1],
                                    ihis[:, HOFF[t] * 8:HOFF[t + 1] * 8],
                                    khi * 128, G1, S1,
                                    queue_num=(2 * t + 1) % NSWQ)

                    # attention weights
                    e = wk.tile([128, kt * HEADS], f32, tag="e")
                    ev = e[:].rearrange("p (k h) -> p k h", h=HEADS)
                    adb = ald1s[:, t * HEADS:(t + 1) * HEADS].unsqueeze(1) \
                        .to_broadcast([128, kt, HEADS])
                    nc.vector.tensor_tensor(ev, gv[:, :, D1:G1], adb, op=ALU.add)
                    nc.vector.scalar_tensor_tensor(
                        e[:], e[:], 0.2, e[:], op0=ALU.mult, op1=ALU.max)
                    exw = wk.tile([128, kt * HEADS], bf16, tag="exw")
                    nc.scalar.activation(exw[:], e[:], AF.Exp)

                    den = wk.tile([128, HEADS], f32, tag="den")
                    nc.vector.tensor_reduce(
                        den[:], exw[:].rearrange("p (k h) -> p h k", h=HEADS),
                        axis=AX.X, op=ALU.add)
                    rec = wk.tile([128, HEADS], f32, tag="rec")
                    nc.vector.reciprocal(rec[:], den[:])

                    # weight rows in place, then identity-scatter on PE
                    gf = gv[:, :, 0:D1].rearrange("p k (h c) -> p k h c", c=HID)
                    exb = exw[:].rearrange("p (k h) -> p k h", h=HEADS) \
                        .unsqueeze(3).to_broadcast([128, kt, HEADS, HID])
                    nc.vector.tensor_mul(gf, gf, exb)

                    ps_c = psp.tile([128, D1], f32, tag="mm")
                    for k in range(kt):
                        nc.tensor.matmul(
                            ps_c[:], lhsT=ids[:], rhs=gv[:, k, 0:D1],
                            start=(k == 0), stop=(k == kt - 1))

                    o1 = wk.tile([128, D1], f32, tag="o1")
                    o1v = o1[:].rearrange("p (h c) -> p h c", c=HID)
                    recb = rec[:].unsqueeze(2).to_broadcast([128, HEADS, HID])
                    nc.vector.tensor_tensor(
                        o1v, ps_c[:].rearrange("p (h c) -> p h c", c=HID),
                        recb, op=ALU.mult)
                    nc.vector.tensor_add(o1[:], o1[:], b1s[:])
                    # elu = max(x,0) + exp(min(x,0)) - 1, fp32, bf16 out
                    zb = zc[:].to_broadcast([128, D1])
                    tn = wk.tile([128, D1], f32, tag="tn")
                    nc.vector.tensor_tensor(tn[:], o1[:], zb, op=ALU.min)
                    nc.scalar.activation(tn[:], tn[:], AF.Exp)
                    o1b = wk.tile([128, D1], bf16, tag="o1b")
                    nc.vector.tensor_tensor(o1[:], o1[:], zb, op=ALU.max)
                    nc.vector.scalar_tensor_tensor(
                        o1b[:], tn[:], -1.0, o1[:], op0=ALU.add, op1=ALU.add)

                    # fused phase D: z2 rows for this tile
                    tts = []
                    for kk in range(DK):
                        ps_t = pstp.tile([128, 128], bf16, tag="tr")
                        nc.tensor.transpose(
                            ps_t[:], o1b[:, kk * 128:(kk + 1) * 128], ids[:])
                        tt = wk.tile([128, 128], bf16, tag=f"tt{kk}")
                        nc.vector.tensor_copy(tt[:], ps_t[:])
                        tts.append(tt)
                    ps_d = pstp.tile([128, TW2], f32, tag="mm2")
                    for kk in range(DK):
                        nc.tensor.matmul(
                            ps_d[:], lhsT=tts[kk][:],
                            rhs=w2s[:, kk * TW2:(kk + 1) * TW2],
                            start=(kk == 0), stop=(kk == DK - 1))
                    hd = wk.tile([128, G2], bf16, tag="hd")
                    nc.vector.tensor_copy(hd[:], ps_d[:, 0:G2])
                    if t == 0:
                        nc.vector.tensor_add(hd[:, NCLS:G2], hd[:, NCLS:G2],
                                             padms[:, 0:1])
                    if t == NT - 1:
                        nc.vector.tensor_add(hd[:, NCLS:G2], hd[:, NCLS:G2],
                                             padms[:, HEADS:HEADS + 1])
                    nc.sync.dma_start(
                        out=loc2[j][tj * 128:(tj + 1) * 128, 0:G2], in_=hd[:])
                    nc.vector.tensor_copy(ald2s[:, t:t + 1],
                                          ps_d[:, G2:G2 + 1])
                    if tj == TPC - 1:
                        with nc.named_scope("l2_ag"):
                            nc.gpsimd.collective_compute(
                                "AllGather", mybir.AluOpType.bypass,
                                replica_groups=rg,
                                ins=[loc2[j][:]], outs=[t2i[j][:]],
                            )
                        with nc.named_scope("l2_restride"):
                            nc.sync.dma_start(
                                out=tab2[j * CHB:(j + 1) * CHB, :],
                                in_=t2i[j][:])

            # ---- phase F: layer-2 edge pass ----
            with nc.named_scope("l2_edges"):
                for t in range(NT):
                    klo, khi, kt = KLO[t], KHI[t], K[t]
                    g = gvp.tile([128, kt * G2], bf16, tag="g2")
                    gv = g[:].rearrange("p (k c) -> p k c", c=G2)
                    _dma_gather_raw(nc, gv[:, 0:klo, :], tab2[0:HALF, 0:G2],
                                    ilos[:, LOFF[t] * 8:LOFF[t + 1] * 8],
                                    klo * 128, G2, S2,
                                    queue_num=(2 * t) % NSWQ)
                    _dma_gather_raw(nc, gv[:, klo:kt, :],
                                    tab2[HIBASE:NPALL, 0:G2],
                                    ihis[:, HOFF[t] * 8:HOFF[t + 1] * 8],
                                    khi * 128, G2, S2,
                                    queue_num=(2 * t + 1) % NSWQ)

                    e2 = wk.tile([128, kt], f32, tag="e2")
                    adb2 = ald2s[:, t:t + 1].to_broadcast([128, kt])
                    nc.vector.tensor_tensor(e2[:], gv[:, :, NCLS], adb2,
                                            op=ALU.add)
                    nc.vector.scalar_tensor_tensor(
                        e2[:], e2[:], 0.2, e2[:], op0=ALU.mult, op1=ALU.max)
                    # exp -> bf16 back into the as2 column slot
                    nc.scalar.activation(gv[:, :, NCLS], e2[:], AF.Exp)

                    exb2 = gv[:, :, NCLS].unsqueeze(2) \
                        .to_broadcast([128, kt, NCLS])
                    nc.vector.tensor_mul(gv[:, :, 0:NCLS], gv[:, :, 0:NCLS],
                                         exb2)
                    ps_f = psp.tile([128, G2], f32, tag="mm")
                    for k in range(kt):
                        nc.tensor.matmul(ps_f[:], lhsT=ids[:],
                                         rhs=gv[:, k, 0:G2],
                                         start=(k == 0), stop=(k == kt - 1))
                    den2 = wk.tile([128, 1], f32, tag="den2")
                    nc.vector.tensor_copy(den2[:], ps_f[:, NCLS:G2])
                    rec2 = wk.tile([128, 1], f32, tag="rec2")
                    nc.vector.reciprocal(rec2[:], den2[:])

                    o2 = wk.tile([128, NCLS], f32, tag="o2")
                    nc.vector.tensor_tensor(o2[:], ps_f[:, 0:NCLS],
                                            rec2[:].to_broadcast([128, NCLS]),
                                            op=ALU.mult)
                    nc.vector.tensor_add(o2[:], o2[:], b2s[:])

                    nrmax = wk.tile([128, 1], f32, tag="nrmax")
                    nc.vector.tensor_reduce(nrmax[:], o2[:], axis=AX.X,
                                            op=ALU.max, negate=True)
                    sh = shs[:, t * NCLS:(t + 1) * NCLS]
                    nc.vector.tensor_tensor(sh, o2[:],
                                            nrmax[:].to_broadcast([128, NCLS]),
                                            op=ALU.add)
                    exs = wk.tile([128, NCLS], f32, tag="exs")
                    nc.scalar.activation(exs[:], sh, AF.Exp,
                                         accum_out=ssums[:, t:t + 1])

            # ---- phase G: batched log + final output ----
            with nc.named_scope("logsoftmax"):
                nc.scalar.activation(lgss[:], ssums[:], AF.Ln)
                for t in range(NT):
                    outf = wk.tile([128, NCLS], f32, tag="outf")
                    nc.vector.tensor_tensor(
                        outf[:], shs[:, t * NCLS:(t + 1) * NCLS],
                        lgss[:, t:t + 1].to_broadcast([128, NCLS]),
                        op=ALU.subtract)
                    nc.sync.dma_start(out=outp[t * 128:(t + 1) * 128, :],
                                      in_=outf[:])

    nc.compile()
    return nc


def _get_program(cfg):
    key = (cfg["KLO"], cfg["KHI"])
    if key not in _BUILD_CACHE:
        _BUILD_CACHE[key] = _build_program(cfg["KLO"], cfg["KHI"])
    return _BUILD_CACHE[key]


def kernel(**inputs):
    cfg, in_maps, node_at = _host_prep(
        inputs["x"], inputs["edge_index"], inputs["W1"], inputs["a_src1"],
        inputs["a_dst1"], inputs["b1"], inputs["W2"], inputs["a_src2"],
        inputs["a_dst2"], inputs["b2"],
    )
    nc = _get_program(cfg)

    from concourse.bass_utils import run_bass_kernel_spmd

    trace = bool(int(os.environ.get("GAT_PROFILE", "0")))
    if trace:
        trace = _register_trace_hook()
    res = run_bass_kernel_spmd(nc, in_maps, list(range(C)), trace=trace)
    if trace and res.exec_time_ns is not None:
        print(f"HW exec time: {res.exec_time_ns} ns", flush=True)

    out = np.empty((N, NCLS), np.float32)
    for c in range(C):
        r = np.asarray(res.results[c]["outp"], np.float32)
        m = node_at[c] >= 0
        out[node_at[c][m]] = r[m]
    return out
